# revision 1
# baseline (speedup 1.0000x reference)
"""Trainium2 Bass kernel for BiLinearLayerV2.

  biLinear[b,f,d] = sum_e feature[b,f,e] * weight[f,e,d]
  out[b,f,g,d]    = biLinear[b,f,d] * feature[b,g,d] * weightLeft[f,g]

Shapes: feature [512,64,32] f32, weight [64,32,32], weightLeft [64,64]
Output: [512,64,64,32] f32 (256 MB) -> heavily store-bound.

Strategy: data-parallel over batch (64 per core x 8 cores). Per core, using
the fused weight tensor WV[f][e',(g,d)] = weight[f,e',d] * weightLeft[f,g]
(host-precomputed in fp64, a pure weight-layout prep):

  PE:  Q_f[b,(g,d)] = sum_e' featT[e',b] * WV_f[e',(g,d)]
                    = biLinear[b,f,d] * weightLeft[f,g]
       one K=32 matmul per f (4 instrs of N=512); four f's run concurrently
       in disjoint PE row/col regions via tile_position.
  DVE: one tensor_mul pass out = Q * feature (PSUM -> SBUF).
  DMA: 1 MB loads / 512 KB stores, 8 KB contiguous per partition.

fp16 variants stream 4x faster through the PE (1 cyc/col vs 4 for fp32) and
halve the WV load; values are pre-scaled by 2^10 to clear fp16's subnormal
range, undone in the final DVE op. "f16x2" splits feature into an fp16
hi+lo pair (feature exact; only WV carries fp16 rounding).
"""

import sys

if "/opt/trn_rl_repo" not in sys.path:
    sys.path.insert(0, "/opt/trn_rl_repo")

import numpy as np

B, F, E = 512, 64, 32
NCORES = 8
BLOC = B // NCORES  # 64
GD = F * E  # 2048
SCALE = 1024  # 2^10 pre-scale keeps fp16 operand values out of the subnormal range

# dtype of the Q-matmul operands:
#   "f32"   exact (rel err ~4e-7), fp32 streams at 4 cyc/col
#   "f16"   fastest (~1e-3 max rel err), 1 cyc/col, halves WV DMA
#   "f16x2" fp16 hi+lo split of feature, two accumulating matmuls:
#           feature exact, only WV rounds -> ~5e-4 max rel err
QDT = "f32"

_cached = {}


def _build_nc(qdt=QDT):
    from contextlib import ExitStack

    import concourse.bass as bass
    import concourse.tile as tile
    from concourse import bacc, mybir

    f32 = mybir.dt.float32
    qdt_my = {
        "f32": mybir.dt.float32,
        "f16": mybir.dt.float16,
        "f16x2": mybir.dt.float16,
    }[qdt]
    fsplit = qdt == "f16x2"
    scaled = qdt in ("f16", "f16x2")
    nterms = 2 if fsplit else 1
    nc = bacc.Bacc("TRN2", target_bir_lowering=False, debug=False)

    featd = nc.dram_tensor("featd", (2 * BLOC, GD), f32, kind="ExternalInput").ap()
    featT = nc.dram_tensor(
        "featT", (128, F * BLOC), qdt_my, kind="ExternalInput"
    ).ap()
    if fsplit:
        featT_lo = nc.dram_tensor(
            "featT_lo", (128, F * BLOC), qdt_my, kind="ExternalInput"
        ).ap()
    wv = nc.dram_tensor("wv", (16, 128, GD), qdt_my, kind="ExternalInput").ap()
    out = nc.dram_tensor("out", (BLOC, F, GD), f32, kind="ExternalOutput").ap()

    with tile.TileContext(nc) as tc, ExitStack() as ctx:
        consts = ctx.enter_context(tc.tile_pool(name="consts", bufs=1))
        featd_t = consts.tile([128, GD], f32)
        nc.scalar.dma_start(featd_t[:], featd)
        featT_t = consts.tile([128, F * BLOC], qdt_my)
        nc.scalar.dma_start(featT_t[:], featT)
        if fsplit:
            featT_lo_t = consts.tile([128, F * BLOC], qdt_my)
            nc.scalar.dma_start(featT_lo_t[:], featT_lo)

        with (
            tc.tile_pool(name="wd", bufs=4) as wdp,
            tc.tile_pool(name="psq", bufs=1, space=bass.MemorySpace.PSUM) as psq,
            tc.tile_pool(name="ot", bufs=6) as otp,
        ):
            for j4 in range(16):
                wdt = wdp.tile([128, GD], qdt_my)
                nc.gpsimd.dma_start(wdt[:], wv[j4])
                # Both s-pairs' matmuls interleaved at term level so all four
                # disjoint PE row/col regions (r=0..3) run concurrently.
                pqs = [
                    psq.tile([128, GD], f32, name=f"pq{s}", tag=f"pq{s}")
                    for s in range(2)
                ]
                for n in range(4):
                    nsl = slice(512 * n, 512 * (n + 1))
                    for s in range(2):
                        for half in range(2):
                            r = 2 * s + half
                            f = 4 * j4 + r
                            rsl = slice(32 * r, 32 * r + 32)
                            fsl = slice(f * BLOC, (f + 1) * BLOC)
                            for t in range(nterms):
                                src = featT_t if t == 0 else featT_lo_t
                                nc.tensor.matmul(
                                    pqs[s][64 * half : 64 * half + 64, nsl],
                                    src[rsl, fsl],
                                    wdt[rsl, nsl],
                                    start=(t == 0),
                                    stop=(t == nterms - 1),
                                    tile_position=(32 * r, 64 * half),
                                )
                for s in range(2):
                    pq = pqs[s]
                    ot = otp.tile([128, GD], f32)
                    if scaled:
                        # out = (Q / S^2) * feature, undoing the fp16 scaling
                        nc.vector.scalar_tensor_tensor(
                            ot[:],
                            pq[:],
                            1.0 / float(SCALE * SCALE),
                            featd_t[:],
                            op0=mybir.AluOpType.mult,
                            op1=mybir.AluOpType.mult,
                        )
                    else:
                        nc.vector.tensor_mul(ot[:], pq[:], featd_t[:])
                    f0 = 4 * j4 + 2 * s
                    eng = nc.sync if s == 0 else nc.scalar
                    eng.dma_start(out[:, f0, :], ot[0:64, :])
                    eng.dma_start(out[:, f0 + 1, :], ot[64:128, :])

    nc.compile()
    return nc


def _get_nc(qdt=QDT):
    if qdt not in _cached:
        _cached[qdt] = _build_nc(qdt)
    return _cached[qdt]


def _host_inputs(feature, weight, weightLeft, qdt=QDT):
    """Per-core input maps. Host work is layout prep of weights/inputs only."""
    feature = np.ascontiguousarray(feature, dtype=np.float32)
    weight = np.ascontiguousarray(weight, dtype=np.float32)
    weightLeft = np.ascontiguousarray(weightLeft, dtype=np.float32)

    # WV[f, e', g, d] = weight[f,e',d] * W[f,g], fused in fp64, grouped so
    # partitions 32r..32r+31 of group j4 hold WV for f = 4*j4 + r.
    wv64 = weight.astype(np.float64)[:, :, None, :] * weightLeft.astype(np.float64)[
        :, None, :, None
    ]  # [F, E, F(g), E(d)]
    if qdt == "f32":
        wv = wv64.astype(np.float32).reshape(16, 4 * E, F * E)
    else:
        wv = (wv64 * SCALE).astype(np.float32).astype(np.float16)
        wv = wv.reshape(16, 4 * E, F * E)
    wv = np.ascontiguousarray(wv)

    in_maps = []
    for c in range(NCORES):
        fc = feature[c * BLOC : (c + 1) * BLOC]  # [64, 64, 32]
        featd = np.ascontiguousarray(
            np.tile(fc.reshape(BLOC, GD), (2, 1))
        )  # [128, 2048]
        ft = np.ascontiguousarray(fc.transpose(2, 1, 0)).reshape(E, F * BLOC)
        m = {"featd": featd}
        if qdt == "f32":
            m["featT"] = np.ascontiguousarray(np.tile(ft, (4, 1)))  # [128, 4096]
        else:
            fts = ft * np.float32(SCALE)
            hi = fts.astype(np.float16)
            m["featT"] = np.ascontiguousarray(np.tile(hi, (4, 1)))
            if qdt == "f16x2":
                lo = (fts - hi.astype(np.float32)).astype(np.float16)
                m["featT_lo"] = np.ascontiguousarray(np.tile(lo, (4, 1)))
        m["wv"] = wv
        in_maps.append(m)
    return in_maps


def _run(in_maps, trace=False, tmpdir=None, qdt=QDT):
    from concourse.bass_utils import run_bass_kernel_spmd

    nc = _get_nc(qdt)
    return run_bass_kernel_spmd(
        nc, in_maps, core_ids=list(range(NCORES)), trace=trace, tmpdir=tmpdir
    )


def kernel(feature, weight, weightLeft):
    in_maps = _host_inputs(feature, weight, weightLeft)
    res = _run(in_maps)
    out = np.concatenate(
        [r["out"].reshape(BLOC, F, F, E) for r in res.results], axis=0
    )
    return out

